# revision 21
# baseline (speedup 1.0000x reference)
"""Adaptive-softmax NLL loss kernel for 8 TRN2 NeuronCores.

Strategy (vocab-parallel tensor parallelism):
  - Each core owns a 1/8 column slice of each cluster's vocab range
    (250 + 1000 + 5032 cols) plus the shared remainder column 50256
    (its exp is scaled by 1/8 on every core so the all-reduced sum is
    exact).
  - Main logits matmul runs in fp8e4m3 with DoubleRow perf mode
    (K packed 2x per PE cell). Inputs are pre-scaled (x*16, w*64) to
    dodge fp8 subnormals; the 1/1024 descale is folded into the
    ScalarE exp's free affine (exp(scale*psum + bias)).
  - ScalarE computes exp over up to 2048-col PSUM spans with a fused
    free-dim accumulate, giving per-cluster partial sum-exp per token.
  - Target logit x[t] . w[y_t] comes from an indirect-DMA gather of
    the owned weight rows (bf16, transposed shard) + multiply/reduce
    on VectorE, masked by ownership.
  - Two 32KB AllReduces (token halves) combine (S0, S1, S2, tgt); the
    first is issued halfway through the last column group so it hides
    under compute.
  - Replicated epilogue: nll = -(cl_sel - lse_cl + tgt - log(S_sel)).

Token layout on chip: token t -> (partition p = t % 128, tile i = t // 128).
"""

import os
import sys
from contextlib import ExitStack

import numpy as np

try:
    import concourse  # noqa: F401
except ImportError:  # pragma: no cover
    for _p in ("/opt/trn_rl_repo", "/root/.axon_site/_ro/trn_rl_repo"):
        if os.path.isdir(_p):
            sys.path.insert(0, _p)
            break

import ml_dtypes

import concourse.bass as bass
import concourse.tile as tile
from concourse import bacc, mybir
from concourse.bass_utils import run_bass_kernel_spmd

BF16 = ml_dtypes.bfloat16
FP8 = ml_dtypes.float8_e4m3

VOCAB, HIDDEN = 50257, 1024
NTOK = 4096          # B * L tokens
NCORES = 8
P = 128
NT = NTOK // P       # 32 token tiles
NTH = NT // 2        # 16 tiles per all-reduce half
B0, B1 = 250, 1250                 # shard-local cluster boundaries
SHARD = 250 + 1000 + 5032 + 1      # 6283 (incl shared col 50256)
WPAD = 6288                        # fp8 W free dim padded to %16
K2 = HIDDEN // 256                 # 4 double-row K chunks
LN8 = float(np.log(8.0))
SX, SW = 16.0, 64.0                # fp8 pre-scales for x and w
INV = 1.0 / (SX * SW)

# column groups (program order; big group last so AR#1 hides under it).
# group 0 computes 3 extra columns (6283:6286 in the padded W8) that hold
# the cluster-head weights; they are excluded from the exp segments.
GROUPS = [(6144, 6286), (0, 2048), (2048, 4096), (4096, 6144)]
# exp/accumulate segments: (lo, hi, acc_col, cluster, biased)
SEGS = [
    (6144, 6282, 5, 2, False),
    (6282, 6283, 6, 2, True),
    (0, 250, 0, 0, False),
    (250, 1250, 1, 1, False),
    (1250, 2048, 2, 2, False),
    (2048, 4096, 3, 2, False),
    (4096, 6144, 4, 2, False),
]
NSEG = 7
REUSE_LDW = os.environ.get("KERNEL_REUSE_LDW", "0") == "1"

LAST_RESULT = None  # BassKernelResults of the most recent run (side channel)


def _build_graph(kc):
    """Build the SPMD Bass graph. kc = number of 128-row K chunks in the
    (possibly bias-augmented) hidden dim; must be even for double-row."""
    assert kc % 2 == 0
    k2n = kc // 2
    hp = kc * P
    nc = bacc.Bacc(
        "TRN2",
        target_bir_lowering=False,
        debug=False,
        enable_asserts=False,
        num_devices=NCORES,
    )
    dt = mybir.dt
    fp = dt.float32
    f8 = dt.float8e4
    Exp = mybir.ActivationFunctionType.Exp
    Ln = mybir.ActivationFunctionType.Ln
    Alu = mybir.AluOpType
    X = mybir.AxisListType.X

    XT8 = nc.declare_dram_parameter("xt8", [P, k2n, 2, NTOK], f8, isOutput=False)
    W8 = nc.declare_dram_parameter("w8", [P, k2n, 2, WPAD], f8, isOutput=False)
    xN = nc.declare_dram_parameter("xn", [NTOK, hp], dt.bfloat16, isOutput=False)
    WT = nc.declare_dram_parameter("wt", [SHARD, hp], dt.bfloat16, isOutput=False)
    YI = nc.declare_dram_parameter("yi", [P, NT], dt.int32, isOutput=False)
    OM = nc.declare_dram_parameter("om", [P, NT], fp, isOutput=False)
    OH = nc.declare_dram_parameter("oh", [P, NT * 3], fp, isOutput=False)
    OUT = nc.declare_dram_parameter("out", [P, NT], fp, isOutput=True)

    segs_by_group = {g: [] for g in range(len(GROUPS))}
    for (lo, hi, acc_col, _cl, biased) in SEGS:
        for g, (g0, g1) in enumerate(GROUPS):
            if lo >= g0 and hi <= g1:
                segs_by_group[g].append((lo, hi, acc_col, biased))
                break
        else:
            raise AssertionError((lo, hi))

    with ExitStack() as ctx:
        tc = ctx.enter_context(tile.TileContext(nc))
        const = ctx.enter_context(tc.tile_pool(name="const", bufs=1))
        wpool = ctx.enter_context(tc.tile_pool(name="wpool", bufs=2))
        expp = ctx.enter_context(tc.tile_pool(name="expp", bufs=2))
        gpool = ctx.enter_context(tc.tile_pool(name="gpool", bufs=2))
        epi = ctx.enter_context(tc.tile_pool(name="epi", bufs=1))
        dram = ctx.enter_context(tc.tile_pool(name="dram", bufs=1, space="DRAM"))

        # ---- resident inputs ----
        xT_sb = const.tile([P, k2n, 2, NTOK], f8)

        def load_xt8_block(b):
            lo, hi = b * 1024, (b + 1) * 1024
            for k in range(k2n):
                for ih in range(2):
                    nc.sync.dma_start(
                        out=xT_sb[:, k, ih, lo:hi], in_=XT8[:, k, ih, lo:hi]
                    )

        load_xt8_block(0)
        yi_sb = const.tile([P, NT], dt.int32)
        nc.sync.dma_start(out=yi_sb[:], in_=YI[:, :])
        om_sb = const.tile([P, NT], fp)
        nc.sync.dma_start(out=om_sb[:], in_=OM[:, :])
        oh_sb = const.tile([P, NT * 3], fp)
        nc.sync.dma_start(out=oh_sb[:], in_=OH[:, :])

        nln8 = const.tile([P, 1], fp)
        nc.vector.memset(nln8[:], -LN8)

        acc = const.tile([P, NT * NSEG], fp)
        tgt_raw = const.tile([P, NT], fp)
        # S_all layout: [half, quantity(S0,S1,S2,tgt), 16 tiles]
        S_all = const.tile([P, 2, 4, NTH], fp)
        R_all = const.tile([P, 2, 4, NTH], fp)
        cl_sb = const.tile([P, NT * 3], fp)

        # ---- target-logit path: gather owned weight rows, fused dot ----
        # (emitted mid main-loop so its DMA traffic doesn't block W8 loads)
        def emit_gather_block():
            for i in range(NT):
                wg = gpool.tile([P, hp], dt.bfloat16, tag="wg", name="wg")
                nc.gpsimd.indirect_dma_start(
                    out=wg[:],
                    out_offset=None,
                    in_=WT[:, :],
                    in_offset=bass.IndirectOffsetOnAxis(ap=yi_sb[:, i:i + 1], axis=0),
                )
                xr = gpool.tile([P, hp], dt.bfloat16, tag="xr", name="xr")
                nc.sync.dma_start(out=xr[:], in_=xN[i * P:(i + 1) * P, :])
                pr = gpool.tile([P, hp], fp, tag="pr", name="pr")
                nc.vector.tensor_mul(out=pr[:], in0=xr[:], in1=wg[:])
                nc.vector.reduce_sum(out=tgt_raw[:, i:i + 1], in_=pr[:], axis=X)

        # ---- main fp8 double-row matmul + fused exp/accumulate ----
        psum = ctx.enter_context(tc.tile_pool(name="psum", bufs=2, space="PSUM"))
        b_in = [
            dram.tile([P, 4 * NTH], fp, name=f"b_in{h}", tag=f"b_in{h}")
            for h in range(2)
        ]
        b_out = [
            dram.tile([P, 4 * NTH], fp, name=f"b_out{h}", tag=f"b_out{h}")
            for h in range(2)
        ]

        def reduce_half(h):
            """Fold acc + tgt partials for token-tile half h and start its
            all-reduce."""
            acc3 = acc[:].rearrange("p (i s) -> p i s", s=NSEG)
            sl = slice(h * NTH, (h + 1) * NTH)
            nc.vector.tensor_copy(out=S_all[:, h, 0, :], in_=acc3[:, sl, 0])
            nc.vector.tensor_copy(out=S_all[:, h, 1, :], in_=acc3[:, sl, 1])
            nc.vector.reduce_sum(out=S_all[:, h, 2, :], in_=acc3[:, sl, 2:NSEG], axis=X)
            nc.vector.tensor_mul(
                out=S_all[:, h, 3, :], in0=tgt_raw[:, sl], in1=om_sb[:, sl]
            )
            nc.gpsimd.dma_start(out=b_in[h][:], in_=S_all[:, h, :, :])
            nc.gpsimd.collective_compute(
                "AllReduce",
                Alu.add,
                replica_groups=[list(range(NCORES))],
                ins=[b_in[h].opt()],
                outs=[b_out[h].opt()],
            )
            nc.gpsimd.dma_start(out=R_all[:, h, :, :], in_=b_out[h][:])

        # ---- per-half epilogue (replicated on all cores) ----
        def emit_epilogue(h):
            hsl = slice(h * NTH, (h + 1) * NTH)      # [P, 16] ranges
            h3 = slice(h * NTH * 3, (h + 1) * NTH * 3)
            ecl = epi.tile([P, NTH * 3], fp, tag=f"ecl{h}", name=f"ecl{h}")
            nc.scalar.activation(out=ecl[:], in_=cl_sb[:, h3], func=Exp)
            sum_cl = epi.tile([P, NTH], fp, tag=f"sum_cl{h}", name=f"sum_cl{h}")
            nc.vector.reduce_sum(
                out=sum_cl[:], in_=ecl[:].rearrange("p (i c) -> p i c", c=3), axis=X
            )
            lse_cl = epi.tile([P, NTH], fp, tag=f"lse_cl{h}", name=f"lse_cl{h}")
            nc.scalar.activation(out=lse_cl[:], in_=sum_cl[:], func=Ln)

            clsel_t = epi.tile([P, NTH * 3], fp, tag=f"clsel{h}", name=f"clsel{h}")
            nc.vector.tensor_mul(out=clsel_t[:], in0=cl_sb[:, h3], in1=oh_sb[:, h3])
            cl_sel = epi.tile([P, NTH], fp, tag=f"cl_sel{h}", name=f"cl_sel{h}")
            nc.vector.reduce_sum(
                out=cl_sel[:], in_=clsel_t[:].rearrange("p (i c) -> p i c", c=3),
                axis=X,
            )

            # R_all[:, h] is [P, 4, NTH]: S_c at [:, c, il]; view as [p, il, c]
            ssel_t = epi.tile([P, NTH * 3], fp, tag=f"ssel{h}", name=f"ssel{h}")
            rview = R_all[:, h, :, :].rearrange("p c il -> p il c")[:, :, 0:3]
            nc.vector.tensor_tensor(
                out=ssel_t[:].rearrange("p (il c) -> p il c", c=3),
                in0=rview,
                in1=oh_sb[:, h3].rearrange("p (il c) -> p il c", c=3),
                op=Alu.mult,
            )
            S_sel = epi.tile([P, NTH], fp, tag=f"S_sel{h}", name=f"S_sel{h}")
            nc.vector.reduce_sum(
                out=S_sel[:], in_=ssel_t[:].rearrange("p (i c) -> p i c", c=3), axis=X
            )
            logS = epi.tile([P, NTH], fp, tag=f"logS{h}", name=f"logS{h}")
            nc.scalar.activation(out=logS[:], in_=S_sel[:], func=Ln)

            t1 = epi.tile([P, NTH], fp, tag=f"t1{h}", name=f"t1{h}")
            nc.vector.tensor_sub(out=t1[:], in0=cl_sel[:], in1=lse_cl[:])
            t2 = epi.tile([P, NTH], fp, tag=f"t2{h}", name=f"t2{h}")
            nc.vector.tensor_sub(out=t2[:], in0=R_all[:, h, 3, :], in1=logS[:])
            t3 = epi.tile([P, NTH], fp, tag=f"t3{h}", name=f"t3{h}")
            nc.vector.tensor_add(out=t3[:], in0=t1[:], in1=t2[:])
            res = epi.tile([P, NTH], fp, tag=f"res{h}", name=f"res{h}")
            nc.vector.tensor_scalar_mul(res[:], t3[:], -1.0)
            nc.sync.dma_start(out=OUT[:, hsl], in_=res[:])

        n_groups = len(GROUPS)
        for g, (g0, g1) in enumerate(GROUPS):
            gw = g1 - g0
            nsub = (gw + 511) // 512
            wt_t = wpool.tile([P, k2n, 2, 2048], f8, tag="w")
            for k in range(k2n):
                for ih in range(2):
                    nc.sync.dma_start(
                        out=wt_t[:, k, ih, :gw], in_=W8[:, k, ih, g0:g0 + gw]
                    )
            if g == 0:
                for b in range(1, 4):
                    load_xt8_block(b)
            for i in range(NT):
                ps = psum.tile([P, 2048], fp)
                for k in range(k2n):
                    for sub in range(nsub):
                        c0 = sub * 512
                        ncols = min(512, gw - c0)
                        mm = nc.tensor.matmul(
                            ps[:, c0:c0 + ncols],
                            lhsT=xT_sb[:, k, :, i * P:(i + 1) * P],
                            rhs=wt_t[:, k, :, c0:c0 + ncols],
                            start=(k == 0),
                            stop=(k == k2n - 1),
                            perf_mode=mybir.MatmulPerfMode.DoubleRow,
                            skip_group_check=True,
                        )
                        if REUSE_LDW and sub > 0:
                            mm.ins.ldweights = False
                if g == 0:
                    # cluster-head logits live in the 3 pad columns
                    nc.vector.tensor_scalar_mul(
                        cl_sb[:, i * 3:(i + 1) * 3], ps[:, 139:142], INV
                    )
                ex = expp.tile([P, 2048], fp, tag="ex")
                for (lo, hi, acc_col, biased) in segs_by_group[g]:
                    nc.scalar.activation(
                        out=ex[:, lo - g0:hi - g0],
                        in_=ps[:, lo - g0:hi - g0],
                        func=Exp,
                        bias=(nln8[:] if biased else 0.0),
                        scale=INV,
                        accum_out=acc[:, i * NSEG + acc_col:i * NSEG + acc_col + 1],
                    )
                if g == n_groups - 1 and i == NTH - 1:
                    reduce_half(0)
                    emit_epilogue(0)
            if g == 1:
                emit_gather_block()
            if g == n_groups - 1:
                reduce_half(1)
                emit_epilogue(1)

    return nc


def _shard_cols(k):
    return np.concatenate(
        [
            np.arange(250 * k, 250 * (k + 1)),
            np.arange(2000 + 1000 * k, 2000 + 1000 * (k + 1)),
            np.arange(10000 + 5032 * k, 10000 + 5032 * (k + 1)),
            np.array([50256]),
        ]
    )


def _tok_layout(v):
    """[4096] vector -> [128, 32] with A[p, i] = v[i*128 + p]."""
    return np.ascontiguousarray(v.reshape(NT, P).T)


def _pack_dr(m, width):
    """[hp, width] -> double-row packed [128, hp//256, 2, width] fp8."""
    hp = m.shape[0]
    return np.ascontiguousarray(
        m.reshape(hp // 256, 2, P, width).transpose(2, 0, 1, 3)
    ).astype(FP8)


def kernel(**inputs):
    global LAST_RESULT
    x = np.asarray(inputs["x"], np.float32)
    y = np.asarray(inputs["y"]).astype(np.int64).reshape(-1)
    cw = np.asarray(inputs["cluster_w"], np.float32)
    cb = np.asarray(inputs["cluster_b"], np.float32).reshape(-1)
    lw = np.asarray(inputs["logits_w"], np.float32)
    lb = np.asarray(inputs["logits_b"], np.float32).reshape(-1)

    x_flat = x[:, :-1].reshape(NTOK, HIDDEN)

    nz_bias = bool(np.any(cb)) or bool(np.any(lb))
    kc = HIDDEN // P + (2 if nz_bias else 0)
    hp = kc * P
    if nz_bias:
        # Fold biases in as extra hidden chunks (2 chunks to keep kc even):
        # x gets a column of ones (rest zeros), weights get the bias row.
        xa = np.zeros((NTOK, hp), np.float32)
        xa[:, :HIDDEN] = x_flat
        xa[:, HIDDEN] = 1.0
        lwa = np.zeros((hp, VOCAB), np.float32)
        lwa[:HIDDEN] = lw
        lwa[HIDDEN] = lb
        cwa = np.zeros((hp, 3), np.float32)
        cwa[:HIDDEN] = cw
        cwa[HIDDEN] = cb
        x_flat, lw, cw = xa, lwa, cwa

    xT = np.ascontiguousarray(x_flat.T)  # [hp, NTOK]
    xt8 = _pack_dr(xT * SX, NTOK)
    xN_bf = x_flat.astype(BF16)

    # onehot over clusters, [128, 32*3] with c contiguous
    c_id = (y >= 2000).astype(np.int64) + (y >= 10000).astype(np.int64)
    oh = np.zeros((NTOK, 3), np.float32)
    oh[np.arange(NTOK), c_id] = 1.0
    oh = np.ascontiguousarray(oh.reshape(NT, P, 3).transpose(1, 0, 2).reshape(P, NT * 3))

    in_maps = []
    for k in range(NCORES):
        cols = _shard_cols(k)
        w_sh = lw[:, cols]  # [hp, SHARD] f32
        wpadded = np.zeros((hp, WPAD), np.float32)
        wpadded[:, :SHARD] = w_sh
        wpadded[:, SHARD:SHARD + 3] = cw
        w8 = _pack_dr(wpadded * SW, WPAD)
        wt_bf = np.ascontiguousarray(w_sh.T).astype(BF16)

        loc = np.zeros(NTOK, np.int64)
        r0 = (y >= 250 * k) & (y < 250 * (k + 1))
        loc[r0] = y[r0] - 250 * k
        r1 = (y >= 2000 + 1000 * k) & (y < 2000 + 1000 * (k + 1))
        loc[r1] = 250 + y[r1] - (2000 + 1000 * k)
        r2 = (y >= 10000 + 5032 * k) & (y < 10000 + 5032 * (k + 1))
        loc[r2] = 1250 + y[r2] - (10000 + 5032 * k)
        own = r0 | r1 | r2
        if k == NCORES - 1:
            r3 = y == VOCAB - 1
            own = own | r3
            loc[r3] = SHARD - 1

        in_maps.append(
            {
                "xt8": xt8,
                "w8": w8,
                        "xn": xN_bf,
                "wt": wt_bf,
                "yi": _tok_layout(loc).astype(np.int32),
                "om": _tok_layout(own.astype(np.float32)),
                "oh": oh,
            }
        )

    nc = _build_graph(kc)
    if not nc.is_finalized():
        nc.finalize()  # bass2jax serializes as-is; Bacc needs alloc_regs etc.
    result = run_bass_kernel_spmd(nc, in_maps, core_ids=list(range(NCORES)))
    LAST_RESULT = result
    out = np.asarray(result.results[0]["out"], np.float32)  # [128, 32]
    return np.ascontiguousarray(out.T).reshape(-1)


# revision 22
# speedup vs baseline: 1.0278x; 1.0278x over previous
"""Adaptive-softmax NLL loss kernel for 8 TRN2 NeuronCores.

Strategy (vocab-parallel tensor parallelism):
  - Each core owns a 1/8 column slice of each cluster's vocab range
    (250 + 1000 + 5032 cols) plus the shared remainder column 50256
    (its exp is scaled by 1/8 on every core so the all-reduced sum is
    exact).
  - Main logits matmul runs in fp8e4m3 with DoubleRow perf mode
    (K packed 2x per PE cell). Inputs are pre-scaled (x*16, w*64) to
    dodge fp8 subnormals; the 1/1024 descale is folded into the
    ScalarE exp's free affine (exp(scale*psum + bias)).
  - ScalarE computes exp over up to 2048-col PSUM spans with a fused
    free-dim accumulate, giving per-cluster partial sum-exp per token.
  - Target logit x[t] . w[y_t] comes from an indirect-DMA gather of
    the owned weight rows (bf16, transposed shard) + multiply/reduce
    on VectorE, masked by ownership.
  - Two 32KB AllReduces (token halves) combine (S0, S1, S2, tgt); the
    first is issued halfway through the last column group so it hides
    under compute.
  - Replicated epilogue: nll = -(cl_sel - lse_cl + tgt - log(S_sel)).

Token layout on chip: token t -> (partition p = t % 128, tile i = t // 128).
"""

import os
import sys
from contextlib import ExitStack

import numpy as np

try:
    import concourse  # noqa: F401
except ImportError:  # pragma: no cover
    for _p in ("/opt/trn_rl_repo", "/root/.axon_site/_ro/trn_rl_repo"):
        if os.path.isdir(_p):
            sys.path.insert(0, _p)
            break

import ml_dtypes

import concourse.bass as bass
import concourse.tile as tile
from concourse import bacc, mybir
from concourse.bass_utils import run_bass_kernel_spmd

BF16 = ml_dtypes.bfloat16
FP8 = ml_dtypes.float8_e4m3

VOCAB, HIDDEN = 50257, 1024
NTOK = 4096          # B * L tokens
NCORES = 8
P = 128
NT = NTOK // P       # 32 token tiles
NTH = NT // 2        # 16 tiles per all-reduce half
B0, B1 = 250, 1250                 # shard-local cluster boundaries
SHARD = 250 + 1000 + 5032 + 1      # 6283 (incl shared col 50256)
WPAD = 6288                        # fp8 W free dim padded to %16
K2 = HIDDEN // 256                 # 4 double-row K chunks
LN8 = float(np.log(8.0))
SX, SW = 16.0, 64.0                # fp8 pre-scales for x and w
INV = 1.0 / (SX * SW)

# column groups (program order; big group last so AR#1 hides under it).
# group 0 computes 3 extra columns (6283:6286 in the padded W8) that hold
# the cluster-head weights; they are excluded from the exp segments.
GROUPS = [(6144, 6286), (0, 2048), (2048, 4096), (4096, 6144)]
# exp/accumulate segments: (lo, hi, acc_col, cluster, biased)
SEGS = [
    (6144, 6282, 5, 2, False),
    (6282, 6283, 6, 2, True),
    (0, 250, 0, 0, False),
    (250, 1250, 1, 1, False),
    (1250, 2048, 2, 2, False),
    (2048, 4096, 3, 2, False),
    (4096, 6144, 4, 2, False),
]
NSEG = 7
REUSE_LDW = os.environ.get("KERNEL_REUSE_LDW", "0") == "1"

LAST_RESULT = None  # BassKernelResults of the most recent run (side channel)


def _build_graph(kc):
    """Build the SPMD Bass graph. kc = number of 128-row K chunks in the
    (possibly bias-augmented) hidden dim; must be even for double-row."""
    assert kc % 2 == 0
    k2n = kc // 2
    hp = kc * P
    nc = bacc.Bacc(
        "TRN2",
        target_bir_lowering=False,
        debug=False,
        enable_asserts=False,
        num_devices=NCORES,
    )
    dt = mybir.dt
    fp = dt.float32
    f8 = dt.float8e4
    Exp = mybir.ActivationFunctionType.Exp
    Ln = mybir.ActivationFunctionType.Ln
    Alu = mybir.AluOpType
    X = mybir.AxisListType.X

    XT8 = nc.declare_dram_parameter("xt8", [P, k2n, 2, NTOK], f8, isOutput=False)
    W8 = nc.declare_dram_parameter("w8", [P, k2n, 2, WPAD], f8, isOutput=False)
    xN = nc.declare_dram_parameter("xn", [NTOK, hp], dt.bfloat16, isOutput=False)
    WT = nc.declare_dram_parameter("wt", [SHARD, hp], dt.bfloat16, isOutput=False)
    YI = nc.declare_dram_parameter("yi", [P, NT], dt.int32, isOutput=False)
    OM = nc.declare_dram_parameter("om", [P, NT], fp, isOutput=False)
    OH = nc.declare_dram_parameter("oh", [P, NT * 3], fp, isOutput=False)
    OUT = nc.declare_dram_parameter("out", [P, NT], fp, isOutput=True)

    segs_by_group = {g: [] for g in range(len(GROUPS))}
    for (lo, hi, acc_col, _cl, biased) in SEGS:
        for g, (g0, g1) in enumerate(GROUPS):
            if lo >= g0 and hi <= g1:
                segs_by_group[g].append((lo, hi, acc_col, biased))
                break
        else:
            raise AssertionError((lo, hi))

    with ExitStack() as ctx:
        tc = ctx.enter_context(tile.TileContext(nc))
        const = ctx.enter_context(tc.tile_pool(name="const", bufs=1))
        wpool = ctx.enter_context(tc.tile_pool(name="wpool", bufs=2))
        expp = ctx.enter_context(tc.tile_pool(name="expp", bufs=2))
        gpool = ctx.enter_context(tc.tile_pool(name="gpool", bufs=2))
        epi = ctx.enter_context(tc.tile_pool(name="epi", bufs=1))
        dram = ctx.enter_context(tc.tile_pool(name="dram", bufs=1, space="DRAM"))

        # ---- resident inputs ----
        xT_sb = const.tile([P, k2n, 2, NTOK], f8)

        def load_xt8_block(b):
            lo, hi = b * 1024, (b + 1) * 1024
            for k in range(k2n):
                for ih in range(2):
                    nc.sync.dma_start(
                        out=xT_sb[:, k, ih, lo:hi], in_=XT8[:, k, ih, lo:hi]
                    )

        load_xt8_block(0)
        yi_sb = const.tile([P, NT], dt.int32)
        nc.sync.dma_start(out=yi_sb[:], in_=YI[:, :])
        om_sb = const.tile([P, NT], fp)
        nc.sync.dma_start(out=om_sb[:], in_=OM[:, :])
        oh_sb = const.tile([P, NT * 3], fp)
        nc.sync.dma_start(out=oh_sb[:], in_=OH[:, :])

        nln8 = const.tile([P, 1], fp)
        nc.vector.memset(nln8[:], -LN8)

        acc = const.tile([P, NT * NSEG], fp)
        tgt_raw = const.tile([P, NT], fp)
        # S_all layout: [half, quantity(S0,S1,S2,tgt), 16 tiles]
        S_all = const.tile([P, 2, 4, NTH], fp)
        R_all = const.tile([P, 2, 4, NTH], fp)
        cl_sb = const.tile([P, NT * 3], fp)

        # ---- target-logit path: gather owned weight rows, fused dot ----
        # (emitted mid main-loop so its DMA traffic doesn't block W8 loads)
        def emit_gather_block():
            for i in range(NT):
                wg = gpool.tile([P, hp], dt.bfloat16, tag="wg", name="wg")
                nc.gpsimd.indirect_dma_start(
                    out=wg[:],
                    out_offset=None,
                    in_=WT[:, :],
                    in_offset=bass.IndirectOffsetOnAxis(ap=yi_sb[:, i:i + 1], axis=0),
                )
                xr = gpool.tile([P, hp], dt.bfloat16, tag="xr", name="xr")
                nc.sync.dma_start(out=xr[:], in_=xN[i * P:(i + 1) * P, :])
                pr = gpool.tile([P, hp], fp, tag="pr", name="pr")
                nc.vector.tensor_mul(out=pr[:], in0=xr[:], in1=wg[:])
                nc.vector.reduce_sum(out=tgt_raw[:, i:i + 1], in_=pr[:], axis=X)

        # ---- main fp8 double-row matmul + fused exp/accumulate ----
        psum = ctx.enter_context(tc.tile_pool(name="psum", bufs=2, space="PSUM"))
        b_in = [
            dram.tile([P, 4 * NTH], fp, name=f"b_in{h}", tag=f"b_in{h}")
            for h in range(2)
        ]
        b_out = [
            dram.tile([P, 4 * NTH], fp, name=f"b_out{h}", tag=f"b_out{h}")
            for h in range(2)
        ]

        def reduce_half(h):
            """Fold acc + tgt partials for token-tile half h and start its
            all-reduce."""
            acc3 = acc[:].rearrange("p (i s) -> p i s", s=NSEG)
            sl = slice(h * NTH, (h + 1) * NTH)
            nc.vector.tensor_copy(out=S_all[:, h, 0, :], in_=acc3[:, sl, 0])
            nc.vector.tensor_copy(out=S_all[:, h, 1, :], in_=acc3[:, sl, 1])
            nc.vector.reduce_sum(out=S_all[:, h, 2, :], in_=acc3[:, sl, 2:NSEG], axis=X)
            nc.vector.tensor_mul(
                out=S_all[:, h, 3, :], in0=tgt_raw[:, sl], in1=om_sb[:, sl]
            )
            nc.gpsimd.dma_start(out=b_in[h][:], in_=S_all[:, h, :, :])
            nc.gpsimd.collective_compute(
                "AllReduce",
                Alu.add,
                replica_groups=[list(range(NCORES))],
                ins=[b_in[h].opt()],
                outs=[b_out[h].opt()],
            )
            nc.gpsimd.dma_start(out=R_all[:, h, :, :], in_=b_out[h][:])

        # ---- epilogue, split so only the AR-dependent suffix is on the
        # critical tail: cl_part = cl_sel - lse_cl precomputes after group 0.
        cl_part = epi.tile([P, NT], fp)

        def emit_cl_part():
            ecl = epi.tile([P, NT * 3], fp)
            nc.scalar.activation(out=ecl[:], in_=cl_sb[:], func=Exp)
            sum_cl = epi.tile([P, NT], fp)
            nc.vector.reduce_sum(
                out=sum_cl[:], in_=ecl[:].rearrange("p (i c) -> p i c", c=3), axis=X
            )
            lse_cl = epi.tile([P, NT], fp)
            nc.scalar.activation(out=lse_cl[:], in_=sum_cl[:], func=Ln)
            clsel_t = epi.tile([P, NT * 3], fp)
            nc.vector.tensor_mul(out=clsel_t[:], in0=cl_sb[:], in1=oh_sb[:])
            cl_sel = epi.tile([P, NT], fp)
            nc.vector.reduce_sum(
                out=cl_sel[:], in_=clsel_t[:].rearrange("p (i c) -> p i c", c=3),
                axis=X,
            )
            nc.vector.tensor_sub(out=cl_part[:], in0=cl_sel[:], in1=lse_cl[:])

        def emit_epilogue(h):
            hsl = slice(h * NTH, (h + 1) * NTH)      # [P, 16] ranges
            h3 = slice(h * NTH * 3, (h + 1) * NTH * 3)
            # R_all[:, h] is [P, 4, NTH]: S_c at [:, c, il]; view as [p, il, c]
            ssel_t = epi.tile([P, NTH * 3], fp, tag=f"ssel{h}", name=f"ssel{h}")
            rview = R_all[:, h, :, :].rearrange("p c il -> p il c")[:, :, 0:3]
            nc.vector.tensor_tensor(
                out=ssel_t[:].rearrange("p (il c) -> p il c", c=3),
                in0=rview,
                in1=oh_sb[:, h3].rearrange("p (il c) -> p il c", c=3),
                op=Alu.mult,
            )
            S_sel = epi.tile([P, NTH], fp, tag=f"S_sel{h}", name=f"S_sel{h}")
            nc.vector.reduce_sum(
                out=S_sel[:], in_=ssel_t[:].rearrange("p (i c) -> p i c", c=3), axis=X
            )
            logS = epi.tile([P, NTH], fp, tag=f"logS{h}", name=f"logS{h}")
            nc.scalar.activation(out=logS[:], in_=S_sel[:], func=Ln)
            t2 = epi.tile([P, NTH], fp, tag=f"t2{h}", name=f"t2{h}")
            nc.vector.tensor_sub(out=t2[:], in0=R_all[:, h, 3, :], in1=logS[:])
            # res = -(cl_part + t2) = (t2 * -1) - cl_part
            res = epi.tile([P, NTH], fp, tag=f"res{h}", name=f"res{h}")
            nc.vector.scalar_tensor_tensor(
                out=res[:], in0=t2[:], scalar=-1.0, in1=cl_part[:, hsl],
                op0=Alu.mult, op1=Alu.subtract,
            )
            nc.sync.dma_start(out=OUT[:, hsl], in_=res[:])

        n_groups = len(GROUPS)
        for g, (g0, g1) in enumerate(GROUPS):
            gw = g1 - g0
            nsub = (gw + 511) // 512
            wt_t = wpool.tile([P, k2n, 2, 2048], f8, tag="w")
            for k in range(k2n):
                for ih in range(2):
                    nc.sync.dma_start(
                        out=wt_t[:, k, ih, :gw], in_=W8[:, k, ih, g0:g0 + gw]
                    )
            if g == 0:
                for b in range(1, 4):
                    load_xt8_block(b)
            for i in range(NT):
                ps = psum.tile([P, 2048], fp)
                for sub in range(nsub):
                    c0 = sub * 512
                    ncols = min(512, gw - c0)
                    for k in range(k2n):
                        nc.tensor.matmul(
                            ps[:, c0:c0 + ncols],
                            lhsT=xT_sb[:, k, :, i * P:(i + 1) * P],
                            rhs=wt_t[:, k, :, c0:c0 + ncols],
                            start=(k == 0),
                            stop=(k == k2n - 1),
                            perf_mode=mybir.MatmulPerfMode.DoubleRow,
                        )
                if g == 0:
                    # cluster-head logits live in the 3 pad columns
                    nc.vector.tensor_scalar_mul(
                        cl_sb[:, i * 3:(i + 1) * 3], ps[:, 139:142], INV
                    )
                ex = expp.tile([P, 2048], fp, tag="ex")
                for (lo, hi, acc_col, biased) in segs_by_group[g]:
                    nc.scalar.activation(
                        out=ex[:, lo - g0:hi - g0],
                        in_=ps[:, lo - g0:hi - g0],
                        func=Exp,
                        bias=(nln8[:] if biased else 0.0),
                        scale=INV,
                        accum_out=acc[:, i * NSEG + acc_col:i * NSEG + acc_col + 1],
                    )
                if g == n_groups - 1 and i == NTH - 1:
                    reduce_half(0)
                    emit_epilogue(0)
            if g == 0:
                emit_cl_part()
            if g == 2:
                emit_gather_block()
            if g == n_groups - 1:
                reduce_half(1)
                emit_epilogue(1)

    return nc


def _shard_cols(k):
    return np.concatenate(
        [
            np.arange(250 * k, 250 * (k + 1)),
            np.arange(2000 + 1000 * k, 2000 + 1000 * (k + 1)),
            np.arange(10000 + 5032 * k, 10000 + 5032 * (k + 1)),
            np.array([50256]),
        ]
    )


def _tok_layout(v):
    """[4096] vector -> [128, 32] with A[p, i] = v[i*128 + p]."""
    return np.ascontiguousarray(v.reshape(NT, P).T)


def _pack_dr(m, width):
    """[hp, width] -> double-row packed [128, hp//256, 2, width] fp8."""
    hp = m.shape[0]
    return np.ascontiguousarray(
        m.reshape(hp // 256, 2, P, width).transpose(2, 0, 1, 3)
    ).astype(FP8)


def kernel(**inputs):
    global LAST_RESULT
    x = np.asarray(inputs["x"], np.float32)
    y = np.asarray(inputs["y"]).astype(np.int64).reshape(-1)
    cw = np.asarray(inputs["cluster_w"], np.float32)
    cb = np.asarray(inputs["cluster_b"], np.float32).reshape(-1)
    lw = np.asarray(inputs["logits_w"], np.float32)
    lb = np.asarray(inputs["logits_b"], np.float32).reshape(-1)

    x_flat = x[:, :-1].reshape(NTOK, HIDDEN)

    nz_bias = bool(np.any(cb)) or bool(np.any(lb))
    kc = HIDDEN // P + (2 if nz_bias else 0)
    hp = kc * P
    if nz_bias:
        # Fold biases in as extra hidden chunks (2 chunks to keep kc even):
        # x gets a column of ones (rest zeros), weights get the bias row.
        xa = np.zeros((NTOK, hp), np.float32)
        xa[:, :HIDDEN] = x_flat
        xa[:, HIDDEN] = 1.0
        lwa = np.zeros((hp, VOCAB), np.float32)
        lwa[:HIDDEN] = lw
        lwa[HIDDEN] = lb
        cwa = np.zeros((hp, 3), np.float32)
        cwa[:HIDDEN] = cw
        cwa[HIDDEN] = cb
        x_flat, lw, cw = xa, lwa, cwa

    xT = np.ascontiguousarray(x_flat.T)  # [hp, NTOK]
    xt8 = _pack_dr(xT * SX, NTOK)
    xN_bf = x_flat.astype(BF16)

    # onehot over clusters, [128, 32*3] with c contiguous
    c_id = (y >= 2000).astype(np.int64) + (y >= 10000).astype(np.int64)
    oh = np.zeros((NTOK, 3), np.float32)
    oh[np.arange(NTOK), c_id] = 1.0
    oh = np.ascontiguousarray(oh.reshape(NT, P, 3).transpose(1, 0, 2).reshape(P, NT * 3))

    in_maps = []
    for k in range(NCORES):
        cols = _shard_cols(k)
        w_sh = lw[:, cols]  # [hp, SHARD] f32
        wpadded = np.zeros((hp, WPAD), np.float32)
        wpadded[:, :SHARD] = w_sh
        wpadded[:, SHARD:SHARD + 3] = cw
        w8 = _pack_dr(wpadded * SW, WPAD)
        wt_bf = np.ascontiguousarray(w_sh.T).astype(BF16)

        loc = np.zeros(NTOK, np.int64)
        r0 = (y >= 250 * k) & (y < 250 * (k + 1))
        loc[r0] = y[r0] - 250 * k
        r1 = (y >= 2000 + 1000 * k) & (y < 2000 + 1000 * (k + 1))
        loc[r1] = 250 + y[r1] - (2000 + 1000 * k)
        r2 = (y >= 10000 + 5032 * k) & (y < 10000 + 5032 * (k + 1))
        loc[r2] = 1250 + y[r2] - (10000 + 5032 * k)
        own = r0 | r1 | r2
        if k == NCORES - 1:
            r3 = y == VOCAB - 1
            own = own | r3
            loc[r3] = SHARD - 1

        in_maps.append(
            {
                "xt8": xt8,
                "w8": w8,
                        "xn": xN_bf,
                "wt": wt_bf,
                "yi": _tok_layout(loc).astype(np.int32),
                "om": _tok_layout(own.astype(np.float32)),
                "oh": oh,
            }
        )

    nc = _build_graph(kc)
    if not nc.is_finalized():
        nc.finalize()  # bass2jax serializes as-is; Bacc needs alloc_regs etc.
    result = run_bass_kernel_spmd(nc, in_maps, core_ids=list(range(NCORES)))
    LAST_RESULT = result
    out = np.asarray(result.results[0]["out"], np.float32)  # [128, 32]
    return np.ascontiguousarray(out.T).reshape(-1)


# revision 23
# speedup vs baseline: 1.0560x; 1.0274x over previous
"""Adaptive-softmax NLL loss kernel for 8 TRN2 NeuronCores.

Strategy (vocab-parallel tensor parallelism):
  - Each core owns a 1/8 column slice of each cluster's vocab range
    (250 + 1000 + 5032 cols) plus the shared remainder column 50256
    (its exp is scaled by 1/8 on every core so the all-reduced sum is
    exact).
  - Main logits matmul runs in fp8e4m3 with DoubleRow perf mode
    (K packed 2x per PE cell). Inputs are pre-scaled (x*16, w*64) to
    dodge fp8 subnormals; the 1/1024 descale is folded into the
    ScalarE exp's free affine (exp(scale*psum + bias)).
  - ScalarE computes exp over up to 2048-col PSUM spans with a fused
    free-dim accumulate, giving per-cluster partial sum-exp per token.
  - Target logit x[t] . w[y_t] comes from an indirect-DMA gather of
    the owned weight rows (bf16, transposed shard) + multiply/reduce
    on VectorE, masked by ownership.
  - Two 32KB AllReduces (token halves) combine (S0, S1, S2, tgt); the
    first is issued halfway through the last column group so it hides
    under compute.
  - Replicated epilogue: nll = -(cl_sel - lse_cl + tgt - log(S_sel)).

Token layout on chip: token t -> (partition p = t % 128, tile i = t // 128).
"""

import os
import sys
from contextlib import ExitStack

import numpy as np

try:
    import concourse  # noqa: F401
except ImportError:  # pragma: no cover
    for _p in ("/opt/trn_rl_repo", "/root/.axon_site/_ro/trn_rl_repo"):
        if os.path.isdir(_p):
            sys.path.insert(0, _p)
            break

import ml_dtypes

import concourse.bass as bass
import concourse.tile as tile
from concourse import bacc, mybir
from concourse.bass_utils import run_bass_kernel_spmd

BF16 = ml_dtypes.bfloat16
FP8 = ml_dtypes.float8_e4m3

VOCAB, HIDDEN = 50257, 1024
NTOK = 4096          # B * L tokens
NCORES = 8
P = 128
NT = NTOK // P       # 32 token tiles
NTH = NT // 2        # 16 tiles per all-reduce half
B0, B1 = 250, 1250                 # shard-local cluster boundaries
SHARD = 250 + 1000 + 5032 + 1      # 6283 (incl shared col 50256)
WPAD = 6288                        # fp8 W free dim padded to %16
K2 = HIDDEN // 256                 # 4 double-row K chunks
LN8 = float(np.log(8.0))
SX, SW = 16.0, 64.0                # fp8 pre-scales for x and w
INV = 1.0 / (SX * SW)

# column groups (program order; big group last so AR#1 hides under it).
# group 0 computes 3 extra columns (6283:6286 in the padded W8) that hold
# the cluster-head weights; they are excluded from the exp segments.
GROUPS = [(6144, 6286), (0, 2048), (2048, 4096), (4096, 6144)]
# exp/accumulate segments: (lo, hi, acc_col, cluster, biased)
SEGS = [
    (6144, 6282, 5, 2, False),
    (6282, 6283, 6, 2, True),
    (0, 250, 0, 0, False),
    (250, 1250, 1, 1, False),
    (1250, 2048, 2, 2, False),
    (2048, 4096, 3, 2, False),
    (4096, 6144, 4, 2, False),
]
NSEG = 7
REUSE_LDW = os.environ.get("KERNEL_REUSE_LDW", "0") == "1"

LAST_RESULT = None  # BassKernelResults of the most recent run (side channel)


def _build_graph(kc):
    """Build the SPMD Bass graph. kc = number of 128-row K chunks in the
    (possibly bias-augmented) hidden dim; must be even for double-row."""
    assert kc % 2 == 0
    k2n = kc // 2
    hp = kc * P
    nc = bacc.Bacc(
        "TRN2",
        target_bir_lowering=False,
        debug=False,
        enable_asserts=False,
        num_devices=NCORES,
    )
    dt = mybir.dt
    fp = dt.float32
    f8 = dt.float8e4
    Exp = mybir.ActivationFunctionType.Exp
    Ln = mybir.ActivationFunctionType.Ln
    Alu = mybir.AluOpType
    X = mybir.AxisListType.X

    XT8 = nc.declare_dram_parameter("xt8", [P, k2n, 2, NTOK], f8, isOutput=False)
    W8 = nc.declare_dram_parameter("w8", [P, k2n, 2, WPAD], f8, isOutput=False)
    xN = nc.declare_dram_parameter("xn", [NTOK, hp], dt.bfloat16, isOutput=False)
    WT = nc.declare_dram_parameter("wt", [SHARD, hp], dt.bfloat16, isOutput=False)
    YI = nc.declare_dram_parameter("yi", [P, NT], dt.int32, isOutput=False)
    OM = nc.declare_dram_parameter("om", [P, NT], fp, isOutput=False)
    OH = nc.declare_dram_parameter("oh", [P, NT * 3], fp, isOutput=False)
    OUT = nc.declare_dram_parameter("out", [P, NT], fp, isOutput=True)

    segs_by_group = {g: [] for g in range(len(GROUPS))}
    for (lo, hi, acc_col, _cl, biased) in SEGS:
        for g, (g0, g1) in enumerate(GROUPS):
            if lo >= g0 and hi <= g1:
                segs_by_group[g].append((lo, hi, acc_col, biased))
                break
        else:
            raise AssertionError((lo, hi))

    with ExitStack() as ctx:
        tc = ctx.enter_context(tile.TileContext(nc))
        const = ctx.enter_context(tc.tile_pool(name="const", bufs=1))
        wpool = ctx.enter_context(tc.tile_pool(name="wpool", bufs=2))
        expp = ctx.enter_context(tc.tile_pool(name="expp", bufs=2))
        gpool = ctx.enter_context(tc.tile_pool(name="gpool", bufs=2))
        epi = ctx.enter_context(tc.tile_pool(name="epi", bufs=1))
        dram = ctx.enter_context(tc.tile_pool(name="dram", bufs=1, space="DRAM"))

        # ---- resident inputs ----
        xT_sb = const.tile([P, k2n, 2, NTOK], f8)

        def load_xt8_block(b):
            lo, hi = b * 1024, (b + 1) * 1024
            nc.sync.dma_start(
                out=xT_sb[:, :, :, lo:hi], in_=XT8[:, :, :, lo:hi]
            )

        load_xt8_block(0)
        yi_sb = const.tile([P, NT], dt.int32)
        nc.sync.dma_start(out=yi_sb[:], in_=YI[:, :])
        om_sb = const.tile([P, NT], fp)
        nc.sync.dma_start(out=om_sb[:], in_=OM[:, :])
        oh_sb = const.tile([P, NT * 3], fp)
        nc.sync.dma_start(out=oh_sb[:], in_=OH[:, :])

        nln8 = const.tile([P, 1], fp)
        nc.vector.memset(nln8[:], -LN8)

        acc = const.tile([P, NT * NSEG], fp)
        tgt_raw = const.tile([P, NT], fp)
        # S_all layout: [half, quantity(S0,S1,S2,tgt), 16 tiles]
        S_all = const.tile([P, 2, 4, NTH], fp)
        R_all = const.tile([P, 2, 4, NTH], fp)
        cl_sb = const.tile([P, NT * 3], fp)

        # ---- target-logit path: gather owned weight rows, fused dot ----
        # (emitted mid main-loop so its DMA traffic doesn't block W8 loads)
        def emit_gather_block():
            for i in range(NT):
                wg = gpool.tile([P, hp], dt.bfloat16, tag="wg", name="wg")
                nc.gpsimd.indirect_dma_start(
                    out=wg[:],
                    out_offset=None,
                    in_=WT[:, :],
                    in_offset=bass.IndirectOffsetOnAxis(ap=yi_sb[:, i:i + 1], axis=0),
                )
                xr = gpool.tile([P, hp], dt.bfloat16, tag="xr", name="xr")
                nc.sync.dma_start(out=xr[:], in_=xN[i * P:(i + 1) * P, :])
                pr = gpool.tile([P, hp], fp, tag="pr", name="pr")
                nc.vector.tensor_mul(out=pr[:], in0=xr[:], in1=wg[:])
                nc.vector.reduce_sum(out=tgt_raw[:, i:i + 1], in_=pr[:], axis=X)

        # ---- main fp8 double-row matmul + fused exp/accumulate ----
        psum = ctx.enter_context(tc.tile_pool(name="psum", bufs=2, space="PSUM"))
        b_in = [
            dram.tile([P, 4 * NTH], fp, name=f"b_in{h}", tag=f"b_in{h}")
            for h in range(2)
        ]
        b_out = [
            dram.tile([P, 4 * NTH], fp, name=f"b_out{h}", tag=f"b_out{h}")
            for h in range(2)
        ]

        def reduce_half(h):
            """Fold acc + tgt partials for token-tile half h and start its
            all-reduce."""
            acc3 = acc[:].rearrange("p (i s) -> p i s", s=NSEG)
            sl = slice(h * NTH, (h + 1) * NTH)
            nc.vector.tensor_copy(out=S_all[:, h, 0, :], in_=acc3[:, sl, 0])
            nc.vector.tensor_copy(out=S_all[:, h, 1, :], in_=acc3[:, sl, 1])
            nc.vector.reduce_sum(out=S_all[:, h, 2, :], in_=acc3[:, sl, 2:NSEG], axis=X)
            nc.vector.tensor_mul(
                out=S_all[:, h, 3, :], in0=tgt_raw[:, sl], in1=om_sb[:, sl]
            )
            nc.gpsimd.dma_start(out=b_in[h][:], in_=S_all[:, h, :, :])
            nc.gpsimd.collective_compute(
                "AllReduce",
                Alu.add,
                replica_groups=[list(range(NCORES))],
                ins=[b_in[h].opt()],
                outs=[b_out[h].opt()],
            )
            nc.gpsimd.dma_start(out=R_all[:, h, :, :], in_=b_out[h][:])

        # ---- epilogue, split so only the AR-dependent suffix is on the
        # critical tail: cl_part = cl_sel - lse_cl precomputes after group 0.
        cl_part = epi.tile([P, NT], fp)

        def emit_cl_part():
            ecl = epi.tile([P, NT * 3], fp)
            nc.scalar.activation(out=ecl[:], in_=cl_sb[:], func=Exp)
            sum_cl = epi.tile([P, NT], fp)
            nc.vector.reduce_sum(
                out=sum_cl[:], in_=ecl[:].rearrange("p (i c) -> p i c", c=3), axis=X
            )
            lse_cl = epi.tile([P, NT], fp)
            nc.scalar.activation(out=lse_cl[:], in_=sum_cl[:], func=Ln)
            clsel_t = epi.tile([P, NT * 3], fp)
            nc.vector.tensor_mul(out=clsel_t[:], in0=cl_sb[:], in1=oh_sb[:])
            cl_sel = epi.tile([P, NT], fp)
            nc.vector.reduce_sum(
                out=cl_sel[:], in_=clsel_t[:].rearrange("p (i c) -> p i c", c=3),
                axis=X,
            )
            nc.vector.tensor_sub(out=cl_part[:], in0=cl_sel[:], in1=lse_cl[:])

        def emit_epilogue(h):
            hsl = slice(h * NTH, (h + 1) * NTH)      # [P, 16] ranges
            h3 = slice(h * NTH * 3, (h + 1) * NTH * 3)
            # R_all[:, h] is [P, 4, NTH]: S_c at [:, c, il]; view as [p, il, c]
            ssel_t = epi.tile([P, NTH * 3], fp, tag=f"ssel{h}", name=f"ssel{h}")
            rview = R_all[:, h, :, :].rearrange("p c il -> p il c")[:, :, 0:3]
            nc.vector.tensor_tensor(
                out=ssel_t[:].rearrange("p (il c) -> p il c", c=3),
                in0=rview,
                in1=oh_sb[:, h3].rearrange("p (il c) -> p il c", c=3),
                op=Alu.mult,
            )
            S_sel = epi.tile([P, NTH], fp, tag=f"S_sel{h}", name=f"S_sel{h}")
            nc.vector.reduce_sum(
                out=S_sel[:], in_=ssel_t[:].rearrange("p (i c) -> p i c", c=3), axis=X
            )
            logS = epi.tile([P, NTH], fp, tag=f"logS{h}", name=f"logS{h}")
            nc.scalar.activation(out=logS[:], in_=S_sel[:], func=Ln)
            t2 = epi.tile([P, NTH], fp, tag=f"t2{h}", name=f"t2{h}")
            nc.vector.tensor_sub(out=t2[:], in0=R_all[:, h, 3, :], in1=logS[:])
            # res = -(cl_part + t2) = (t2 * -1) - cl_part
            res = epi.tile([P, NTH], fp, tag=f"res{h}", name=f"res{h}")
            nc.vector.scalar_tensor_tensor(
                out=res[:], in0=t2[:], scalar=-1.0, in1=cl_part[:, hsl],
                op0=Alu.mult, op1=Alu.subtract,
            )
            nc.sync.dma_start(out=OUT[:, hsl], in_=res[:])

        n_groups = len(GROUPS)
        for g, (g0, g1) in enumerate(GROUPS):
            gw = g1 - g0
            nsub = (gw + 511) // 512
            wt_t = wpool.tile([P, k2n, 2, 2048], f8, tag="w")
            nc.sync.dma_start(
                out=wt_t[:, :, :, :gw], in_=W8[:, :, :, g0:g0 + gw]
            )
            if g == 0:
                for b in range(1, 4):
                    load_xt8_block(b)
            for i in range(NT):
                ps = psum.tile([P, 2048], fp)
                for sub in range(nsub):
                    c0 = sub * 512
                    ncols = min(512, gw - c0)
                    for k in range(k2n):
                        nc.tensor.matmul(
                            ps[:, c0:c0 + ncols],
                            lhsT=xT_sb[:, k, :, i * P:(i + 1) * P],
                            rhs=wt_t[:, k, :, c0:c0 + ncols],
                            start=(k == 0),
                            stop=(k == k2n - 1),
                            perf_mode=mybir.MatmulPerfMode.DoubleRow,
                        )
                if g == 0:
                    # cluster-head logits live in the 3 pad columns
                    nc.vector.tensor_scalar_mul(
                        cl_sb[:, i * 3:(i + 1) * 3], ps[:, 139:142], INV
                    )
                ex = expp.tile([P, 2048], fp, tag="ex")
                for (lo, hi, acc_col, biased) in segs_by_group[g]:
                    nc.scalar.activation(
                        out=ex[:, lo - g0:hi - g0],
                        in_=ps[:, lo - g0:hi - g0],
                        func=Exp,
                        bias=(nln8[:] if biased else 0.0),
                        scale=INV,
                        accum_out=acc[:, i * NSEG + acc_col:i * NSEG + acc_col + 1],
                    )
                if g == n_groups - 1 and i == NTH - 1:
                    reduce_half(0)
                    emit_epilogue(0)
            if g == 0:
                emit_cl_part()
            if g == 2:
                emit_gather_block()
            if g == n_groups - 1:
                reduce_half(1)
                emit_epilogue(1)

    return nc


def _shard_cols(k):
    return np.concatenate(
        [
            np.arange(250 * k, 250 * (k + 1)),
            np.arange(2000 + 1000 * k, 2000 + 1000 * (k + 1)),
            np.arange(10000 + 5032 * k, 10000 + 5032 * (k + 1)),
            np.array([50256]),
        ]
    )


def _tok_layout(v):
    """[4096] vector -> [128, 32] with A[p, i] = v[i*128 + p]."""
    return np.ascontiguousarray(v.reshape(NT, P).T)


def _pack_dr(m, width):
    """[hp, width] -> double-row packed [128, hp//256, 2, width] fp8."""
    hp = m.shape[0]
    return np.ascontiguousarray(
        m.reshape(hp // 256, 2, P, width).transpose(2, 0, 1, 3)
    ).astype(FP8)


def kernel(**inputs):
    global LAST_RESULT
    x = np.asarray(inputs["x"], np.float32)
    y = np.asarray(inputs["y"]).astype(np.int64).reshape(-1)
    cw = np.asarray(inputs["cluster_w"], np.float32)
    cb = np.asarray(inputs["cluster_b"], np.float32).reshape(-1)
    lw = np.asarray(inputs["logits_w"], np.float32)
    lb = np.asarray(inputs["logits_b"], np.float32).reshape(-1)

    x_flat = x[:, :-1].reshape(NTOK, HIDDEN)

    nz_bias = bool(np.any(cb)) or bool(np.any(lb))
    kc = HIDDEN // P + (2 if nz_bias else 0)
    hp = kc * P
    if nz_bias:
        # Fold biases in as extra hidden chunks (2 chunks to keep kc even):
        # x gets a column of ones (rest zeros), weights get the bias row.
        xa = np.zeros((NTOK, hp), np.float32)
        xa[:, :HIDDEN] = x_flat
        xa[:, HIDDEN] = 1.0
        lwa = np.zeros((hp, VOCAB), np.float32)
        lwa[:HIDDEN] = lw
        lwa[HIDDEN] = lb
        cwa = np.zeros((hp, 3), np.float32)
        cwa[:HIDDEN] = cw
        cwa[HIDDEN] = cb
        x_flat, lw, cw = xa, lwa, cwa

    xT = np.ascontiguousarray(x_flat.T)  # [hp, NTOK]
    xt8 = _pack_dr(xT * SX, NTOK)
    xN_bf = x_flat.astype(BF16)

    # onehot over clusters, [128, 32*3] with c contiguous
    c_id = (y >= 2000).astype(np.int64) + (y >= 10000).astype(np.int64)
    oh = np.zeros((NTOK, 3), np.float32)
    oh[np.arange(NTOK), c_id] = 1.0
    oh = np.ascontiguousarray(oh.reshape(NT, P, 3).transpose(1, 0, 2).reshape(P, NT * 3))

    in_maps = []
    for k in range(NCORES):
        cols = _shard_cols(k)
        w_sh = lw[:, cols]  # [hp, SHARD] f32
        wpadded = np.zeros((hp, WPAD), np.float32)
        wpadded[:, :SHARD] = w_sh
        wpadded[:, SHARD:SHARD + 3] = cw
        w8 = _pack_dr(wpadded * SW, WPAD)
        wt_bf = np.ascontiguousarray(w_sh.T).astype(BF16)

        loc = np.zeros(NTOK, np.int64)
        r0 = (y >= 250 * k) & (y < 250 * (k + 1))
        loc[r0] = y[r0] - 250 * k
        r1 = (y >= 2000 + 1000 * k) & (y < 2000 + 1000 * (k + 1))
        loc[r1] = 250 + y[r1] - (2000 + 1000 * k)
        r2 = (y >= 10000 + 5032 * k) & (y < 10000 + 5032 * (k + 1))
        loc[r2] = 1250 + y[r2] - (10000 + 5032 * k)
        own = r0 | r1 | r2
        if k == NCORES - 1:
            r3 = y == VOCAB - 1
            own = own | r3
            loc[r3] = SHARD - 1

        in_maps.append(
            {
                "xt8": xt8,
                "w8": w8,
                        "xn": xN_bf,
                "wt": wt_bf,
                "yi": _tok_layout(loc).astype(np.int32),
                "om": _tok_layout(own.astype(np.float32)),
                "oh": oh,
            }
        )

    nc = _build_graph(kc)
    if not nc.is_finalized():
        nc.finalize()  # bass2jax serializes as-is; Bacc needs alloc_regs etc.
    result = run_bass_kernel_spmd(nc, in_maps, core_ids=list(range(NCORES)))
    LAST_RESULT = result
    out = np.asarray(result.results[0]["out"], np.float32)  # [128, 32]
    return np.ascontiguousarray(out.T).reshape(-1)


# revision 24
# speedup vs baseline: 1.0570x; 1.0010x over previous
"""Adaptive-softmax NLL loss kernel for 8 TRN2 NeuronCores.

Strategy (vocab-parallel tensor parallelism):
  - Each core owns a 1/8 column slice of each cluster's vocab range
    (250 + 1000 + 5032 cols) plus the shared remainder column 50256
    (its exp is scaled by 1/8 on every core so the all-reduced sum is
    exact).
  - Main logits matmul runs in fp8e4m3 with DoubleRow perf mode
    (K packed 2x per PE cell). Inputs are pre-scaled (x*16, w*64) to
    dodge fp8 subnormals; the 1/1024 descale is folded into the
    ScalarE exp's free affine (exp(scale*psum + bias)).
  - ScalarE computes exp over up to 2048-col PSUM spans with a fused
    free-dim accumulate, giving per-cluster partial sum-exp per token.
  - Target logit x[t] . w[y_t] comes from an indirect-DMA gather of
    the owned weight rows (bf16, transposed shard) + multiply/reduce
    on VectorE, masked by ownership.
  - Two 32KB AllReduces (token halves) combine (S0, S1, S2, tgt); the
    first is issued halfway through the last column group so it hides
    under compute.
  - Replicated epilogue: nll = -(cl_sel - lse_cl + tgt - log(S_sel)).

Token layout on chip: token t -> (partition p = t % 128, tile i = t // 128).
"""

import os
import sys
from contextlib import ExitStack

import numpy as np

try:
    import concourse  # noqa: F401
except ImportError:  # pragma: no cover
    for _p in ("/opt/trn_rl_repo", "/root/.axon_site/_ro/trn_rl_repo"):
        if os.path.isdir(_p):
            sys.path.insert(0, _p)
            break

import ml_dtypes

import concourse.bass as bass
import concourse.tile as tile
from concourse import bacc, mybir
from concourse.bass_utils import run_bass_kernel_spmd

BF16 = ml_dtypes.bfloat16
FP8 = ml_dtypes.float8_e4m3

VOCAB, HIDDEN = 50257, 1024
NTOK = 4096          # B * L tokens
NCORES = 8
P = 128
NT = NTOK // P       # 32 token tiles
NTH = NT // 2        # 16 tiles per all-reduce half
B0, B1 = 250, 1250                 # shard-local cluster boundaries
SHARD = 250 + 1000 + 5032 + 1      # 6283 (incl shared col 50256)
WPAD = 6288                        # fp8 W free dim padded to %16
K2 = HIDDEN // 256                 # 4 double-row K chunks
LN8 = float(np.log(8.0))
SX, SW = 16.0, 64.0                # fp8 pre-scales for x and w
INV = 1.0 / (SX * SW)

# column groups (program order; big group last so AR#1 hides under it).
# group 0 computes 3 extra columns (6283:6286 in the padded W8) that hold
# the cluster-head weights; they are excluded from the exp segments.
GROUPS = [(6144, 6286), (0, 2048), (2048, 4096), (4096, 6144)]
# exp/accumulate segments: (lo, hi, acc_col, cluster, biased)
SEGS = [
    (6144, 6282, 5, 2, False),
    (6282, 6283, 6, 2, True),
    (0, 250, 0, 0, False),
    (250, 1250, 1, 1, False),
    (1250, 2048, 2, 2, False),
    (2048, 4096, 3, 2, False),
    (4096, 6144, 4, 2, False),
]
NSEG = 7

LAST_RESULT = None  # BassKernelResults of the most recent run (side channel)


def _ensure_ntff_hook():
    """bass_utils' trace path imports antenv.axon_hooks, which the trimmed
    agent image lacks. Register a shim (ctypes NTFF hook if available, else
    None so tracing is skipped gracefully)."""
    try:
        import antenv.axon_hooks  # noqa: F401
        return
    except ImportError:
        pass
    hook = None
    try:
        if "/root/.axon_site" not in sys.path and os.path.isdir("/root/.axon_site"):
            sys.path.append("/root/.axon_site")
        from trn_agent_boot.trn_boot import _ntff_profile_via_ctypes
        hook = _ntff_profile_via_ctypes("/opt/axon/libaxon_pjrt.so")
    except Exception:
        hook = None
    import types

    import antenv

    m = types.ModuleType("antenv.axon_hooks")
    m.get_axon_ntff_profile_hook = lambda _hook=hook: _hook
    m.set_axon_ntff_profile_hook = lambda h: None
    sys.modules["antenv.axon_hooks"] = m
    antenv.axon_hooks = m


def _build_graph(kc):
    """Build the SPMD Bass graph. kc = number of 128-row K chunks in the
    (possibly bias-augmented) hidden dim; must be even for double-row."""
    assert kc % 2 == 0
    k2n = kc // 2
    hp = kc * P
    nc = bacc.Bacc(
        "TRN2",
        target_bir_lowering=False,
        debug=False,
        enable_asserts=False,
        num_devices=NCORES,
    )
    dt = mybir.dt
    fp = dt.float32
    f8 = dt.float8e4
    Exp = mybir.ActivationFunctionType.Exp
    Ln = mybir.ActivationFunctionType.Ln
    Alu = mybir.AluOpType
    X = mybir.AxisListType.X

    XT8 = nc.declare_dram_parameter("xt8", [P, k2n, 2, NTOK], f8, isOutput=False)
    W8 = nc.declare_dram_parameter("w8", [P, k2n, 2, WPAD], f8, isOutput=False)
    xN = nc.declare_dram_parameter("xn", [NTOK, hp], dt.bfloat16, isOutput=False)
    WT = nc.declare_dram_parameter("wt", [SHARD, hp], dt.bfloat16, isOutput=False)
    YI = nc.declare_dram_parameter("yi", [P, NT], dt.int32, isOutput=False)
    OM = nc.declare_dram_parameter("om", [P, NT], fp, isOutput=False)
    OH = nc.declare_dram_parameter("oh", [P, NT * 3], fp, isOutput=False)
    OUT = nc.declare_dram_parameter("out", [P, NT], fp, isOutput=True)

    segs_by_group = {g: [] for g in range(len(GROUPS))}
    for (lo, hi, acc_col, _cl, biased) in SEGS:
        for g, (g0, g1) in enumerate(GROUPS):
            if lo >= g0 and hi <= g1:
                segs_by_group[g].append((lo, hi, acc_col, biased))
                break
        else:
            raise AssertionError((lo, hi))

    with ExitStack() as ctx:
        tc = ctx.enter_context(tile.TileContext(nc))
        const = ctx.enter_context(tc.tile_pool(name="const", bufs=1))
        wpool = ctx.enter_context(tc.tile_pool(name="wpool", bufs=2))
        expp = ctx.enter_context(tc.tile_pool(name="expp", bufs=2))
        gpool = ctx.enter_context(tc.tile_pool(name="gpool", bufs=2))
        epi = ctx.enter_context(tc.tile_pool(name="epi", bufs=1))
        dram = ctx.enter_context(tc.tile_pool(name="dram", bufs=1, space="DRAM"))

        # ---- resident inputs ----
        xT_sb = const.tile([P, k2n, 2, NTOK], f8)

        def load_xt8_block(b):
            lo, hi = b * 1024, (b + 1) * 1024
            nc.sync.dma_start(
                out=xT_sb[:, :, :, lo:hi], in_=XT8[:, :, :, lo:hi]
            )

        load_xt8_block(0)
        yi_sb = const.tile([P, NT], dt.int32)
        nc.sync.dma_start(out=yi_sb[:], in_=YI[:, :])
        om_sb = const.tile([P, NT], fp)
        nc.sync.dma_start(out=om_sb[:], in_=OM[:, :])
        oh_sb = const.tile([P, NT * 3], fp)
        nc.sync.dma_start(out=oh_sb[:], in_=OH[:, :])

        nln8 = const.tile([P, 1], fp)
        nc.vector.memset(nln8[:], -LN8)

        acc = const.tile([P, NT * NSEG], fp)
        tgt_raw = const.tile([P, NT], fp)
        # S_all layout: [half, quantity(S0,S1,S2,tgt), 16 tiles]
        S_all = const.tile([P, 2, 4, NTH], fp)
        R_all = const.tile([P, 2, 4, NTH], fp)
        cl_sb = const.tile([P, NT * 3], fp)

        # ---- target-logit path: gather owned weight rows, fused dot ----
        # (emitted mid main-loop so its DMA traffic doesn't block W8 loads)
        def emit_gather_block():
            for i in range(NT):
                wg = gpool.tile([P, hp], dt.bfloat16, tag="wg", name="wg")
                nc.gpsimd.indirect_dma_start(
                    out=wg[:],
                    out_offset=None,
                    in_=WT[:, :],
                    in_offset=bass.IndirectOffsetOnAxis(ap=yi_sb[:, i:i + 1], axis=0),
                )
                xr = gpool.tile([P, hp], dt.bfloat16, tag="xr", name="xr")
                nc.sync.dma_start(out=xr[:], in_=xN[i * P:(i + 1) * P, :])
                pr = gpool.tile([P, hp], fp, tag="pr", name="pr")
                nc.vector.tensor_mul(out=pr[:], in0=xr[:], in1=wg[:])
                nc.vector.reduce_sum(out=tgt_raw[:, i:i + 1], in_=pr[:], axis=X)

        # ---- main fp8 double-row matmul + fused exp/accumulate ----
        psum = ctx.enter_context(tc.tile_pool(name="psum", bufs=2, space="PSUM"))
        b_in = [
            dram.tile([P, 4 * NTH], fp, name=f"b_in{h}", tag=f"b_in{h}")
            for h in range(2)
        ]
        b_out = [
            dram.tile([P, 4 * NTH], fp, name=f"b_out{h}", tag=f"b_out{h}")
            for h in range(2)
        ]

        def reduce_half(h):
            """Fold acc + tgt partials for token-tile half h and start its
            all-reduce."""
            acc3 = acc[:].rearrange("p (i s) -> p i s", s=NSEG)
            sl = slice(h * NTH, (h + 1) * NTH)
            nc.vector.tensor_copy(out=S_all[:, h, 0, :], in_=acc3[:, sl, 0])
            nc.vector.tensor_copy(out=S_all[:, h, 1, :], in_=acc3[:, sl, 1])
            nc.vector.reduce_sum(out=S_all[:, h, 2, :], in_=acc3[:, sl, 2:NSEG], axis=X)
            nc.vector.tensor_mul(
                out=S_all[:, h, 3, :], in0=tgt_raw[:, sl], in1=om_sb[:, sl]
            )
            nc.gpsimd.dma_start(out=b_in[h][:], in_=S_all[:, h, :, :])
            nc.gpsimd.collective_compute(
                "AllReduce",
                Alu.add,
                replica_groups=[list(range(NCORES))],
                ins=[b_in[h].opt()],
                outs=[b_out[h].opt()],
            )
            nc.gpsimd.dma_start(out=R_all[:, h, :, :], in_=b_out[h][:])

        # ---- epilogue, split so only the AR-dependent suffix is on the
        # critical tail: cl_part = cl_sel - lse_cl precomputes after group 0.
        cl_part = epi.tile([P, NT], fp)

        def emit_cl_part():
            ecl = epi.tile([P, NT * 3], fp)
            nc.scalar.activation(out=ecl[:], in_=cl_sb[:], func=Exp)
            sum_cl = epi.tile([P, NT], fp)
            nc.vector.reduce_sum(
                out=sum_cl[:], in_=ecl[:].rearrange("p (i c) -> p i c", c=3), axis=X
            )
            lse_cl = epi.tile([P, NT], fp)
            nc.scalar.activation(out=lse_cl[:], in_=sum_cl[:], func=Ln)
            clsel_t = epi.tile([P, NT * 3], fp)
            nc.vector.tensor_mul(out=clsel_t[:], in0=cl_sb[:], in1=oh_sb[:])
            cl_sel = epi.tile([P, NT], fp)
            nc.vector.reduce_sum(
                out=cl_sel[:], in_=clsel_t[:].rearrange("p (i c) -> p i c", c=3),
                axis=X,
            )
            nc.vector.tensor_sub(out=cl_part[:], in0=cl_sel[:], in1=lse_cl[:])

        def emit_epilogue(h):
            hsl = slice(h * NTH, (h + 1) * NTH)      # [P, 16] ranges
            h3 = slice(h * NTH * 3, (h + 1) * NTH * 3)
            # R_all[:, h] is [P, 4, NTH]: S_c at [:, c, il]; view as [p, il, c]
            ssel_t = epi.tile([P, NTH * 3], fp, tag=f"ssel{h}", name=f"ssel{h}")
            rview = R_all[:, h, :, :].rearrange("p c il -> p il c")[:, :, 0:3]
            nc.vector.tensor_tensor(
                out=ssel_t[:].rearrange("p (il c) -> p il c", c=3),
                in0=rview,
                in1=oh_sb[:, h3].rearrange("p (il c) -> p il c", c=3),
                op=Alu.mult,
            )
            S_sel = epi.tile([P, NTH], fp, tag=f"S_sel{h}", name=f"S_sel{h}")
            nc.vector.reduce_sum(
                out=S_sel[:], in_=ssel_t[:].rearrange("p (i c) -> p i c", c=3), axis=X
            )
            logS = epi.tile([P, NTH], fp, tag=f"logS{h}", name=f"logS{h}")
            nc.scalar.activation(out=logS[:], in_=S_sel[:], func=Ln)
            t2 = epi.tile([P, NTH], fp, tag=f"t2{h}", name=f"t2{h}")
            nc.vector.tensor_sub(out=t2[:], in0=R_all[:, h, 3, :], in1=logS[:])
            # res = -(cl_part + t2) = (t2 * -1) - cl_part
            res = epi.tile([P, NTH], fp, tag=f"res{h}", name=f"res{h}")
            nc.vector.scalar_tensor_tensor(
                out=res[:], in0=t2[:], scalar=-1.0, in1=cl_part[:, hsl],
                op0=Alu.mult, op1=Alu.subtract,
            )
            nc.sync.dma_start(out=OUT[:, hsl], in_=res[:])

        n_groups = len(GROUPS)
        for g, (g0, g1) in enumerate(GROUPS):
            gw = g1 - g0
            nsub = (gw + 511) // 512
            wt_t = wpool.tile([P, k2n, 2, 2048], f8, tag="w")
            nc.sync.dma_start(
                out=wt_t[:, :, :, :gw], in_=W8[:, :, :, g0:g0 + gw]
            )
            if g == 0:
                for b in range(1, 4):
                    load_xt8_block(b)
            for i in range(NT):
                ps = psum.tile([P, 2048], fp)
                for sub in range(nsub):
                    c0 = sub * 512
                    ncols = min(512, gw - c0)
                    for k in range(k2n):
                        nc.tensor.matmul(
                            ps[:, c0:c0 + ncols],
                            lhsT=xT_sb[:, k, :, i * P:(i + 1) * P],
                            rhs=wt_t[:, k, :, c0:c0 + ncols],
                            start=(k == 0),
                            stop=(k == k2n - 1),
                            perf_mode=mybir.MatmulPerfMode.DoubleRow,
                        )
                if g == 0:
                    # cluster-head logits live in the 3 pad columns
                    nc.vector.tensor_scalar_mul(
                        cl_sb[:, i * 3:(i + 1) * 3], ps[:, 139:142], INV
                    )
                ex = expp.tile([P, 2048], fp, tag="ex")
                for (lo, hi, acc_col, biased) in segs_by_group[g]:
                    nc.scalar.activation(
                        out=ex[:, lo - g0:hi - g0],
                        in_=ps[:, lo - g0:hi - g0],
                        func=Exp,
                        bias=(nln8[:] if biased else 0.0),
                        scale=INV,
                        accum_out=acc[:, i * NSEG + acc_col:i * NSEG + acc_col + 1],
                    )
                if g == n_groups - 1 and i == NTH - 1:
                    reduce_half(0)
                    emit_epilogue(0)
            if g == 0:
                emit_cl_part()
            if g == 2:
                emit_gather_block()
            if g == n_groups - 1:
                reduce_half(1)
                emit_epilogue(1)

    return nc


def _shard_cols(k):
    return np.concatenate(
        [
            np.arange(250 * k, 250 * (k + 1)),
            np.arange(2000 + 1000 * k, 2000 + 1000 * (k + 1)),
            np.arange(10000 + 5032 * k, 10000 + 5032 * (k + 1)),
            np.array([50256]),
        ]
    )


def _tok_layout(v):
    """[4096] vector -> [128, 32] with A[p, i] = v[i*128 + p]."""
    return np.ascontiguousarray(v.reshape(NT, P).T)


def _pack_dr(m, width):
    """[hp, width] -> double-row packed [128, hp//256, 2, width] fp8."""
    hp = m.shape[0]
    return np.ascontiguousarray(
        m.reshape(hp // 256, 2, P, width).transpose(2, 0, 1, 3)
    ).astype(FP8)


def kernel(**inputs):
    global LAST_RESULT
    x = np.asarray(inputs["x"], np.float32)
    y = np.asarray(inputs["y"]).astype(np.int64).reshape(-1)
    cw = np.asarray(inputs["cluster_w"], np.float32)
    cb = np.asarray(inputs["cluster_b"], np.float32).reshape(-1)
    lw = np.asarray(inputs["logits_w"], np.float32)
    lb = np.asarray(inputs["logits_b"], np.float32).reshape(-1)

    x_flat = x[:, :-1].reshape(NTOK, HIDDEN)

    nz_bias = bool(np.any(cb)) or bool(np.any(lb))
    kc = HIDDEN // P + (2 if nz_bias else 0)
    hp = kc * P
    if nz_bias:
        # Fold biases in as extra hidden chunks (2 chunks to keep kc even):
        # x gets a column of ones (rest zeros), weights get the bias row.
        xa = np.zeros((NTOK, hp), np.float32)
        xa[:, :HIDDEN] = x_flat
        xa[:, HIDDEN] = 1.0
        lwa = np.zeros((hp, VOCAB), np.float32)
        lwa[:HIDDEN] = lw
        lwa[HIDDEN] = lb
        cwa = np.zeros((hp, 3), np.float32)
        cwa[:HIDDEN] = cw
        cwa[HIDDEN] = cb
        x_flat, lw, cw = xa, lwa, cwa

    xT = np.ascontiguousarray(x_flat.T)  # [hp, NTOK]
    xt8 = _pack_dr(xT * SX, NTOK)
    xN_bf = x_flat.astype(BF16)

    # onehot over clusters, [128, 32*3] with c contiguous
    c_id = (y >= 2000).astype(np.int64) + (y >= 10000).astype(np.int64)
    oh = np.zeros((NTOK, 3), np.float32)
    oh[np.arange(NTOK), c_id] = 1.0
    oh = np.ascontiguousarray(oh.reshape(NT, P, 3).transpose(1, 0, 2).reshape(P, NT * 3))

    in_maps = []
    for k in range(NCORES):
        cols = _shard_cols(k)
        w_sh = lw[:, cols]  # [hp, SHARD] f32
        wpadded = np.zeros((hp, WPAD), np.float32)
        wpadded[:, :SHARD] = w_sh
        wpadded[:, SHARD:SHARD + 3] = cw
        w8 = _pack_dr(wpadded * SW, WPAD)
        wt_bf = np.ascontiguousarray(w_sh.T).astype(BF16)

        loc = np.zeros(NTOK, np.int64)
        r0 = (y >= 250 * k) & (y < 250 * (k + 1))
        loc[r0] = y[r0] - 250 * k
        r1 = (y >= 2000 + 1000 * k) & (y < 2000 + 1000 * (k + 1))
        loc[r1] = 250 + y[r1] - (2000 + 1000 * k)
        r2 = (y >= 10000 + 5032 * k) & (y < 10000 + 5032 * (k + 1))
        loc[r2] = 1250 + y[r2] - (10000 + 5032 * k)
        own = r0 | r1 | r2
        if k == NCORES - 1:
            r3 = y == VOCAB - 1
            own = own | r3
            loc[r3] = SHARD - 1

        in_maps.append(
            {
                "xt8": xt8,
                "w8": w8,
                        "xn": xN_bf,
                "wt": wt_bf,
                "yi": _tok_layout(loc).astype(np.int32),
                "om": _tok_layout(own.astype(np.float32)),
                "oh": oh,
            }
        )

    _ensure_ntff_hook()
    nc = _build_graph(kc)
    if not nc.is_finalized():
        nc.finalize()  # bass2jax serializes as-is; Bacc needs alloc_regs etc.
    result = run_bass_kernel_spmd(nc, in_maps, core_ids=list(range(NCORES)))
    LAST_RESULT = result
    out = np.asarray(result.results[0]["out"], np.float32)  # [128, 32]
    return np.ascontiguousarray(out.T).reshape(-1)
